# revision 36
# baseline (speedup 1.0000x reference)
"""Trainium2 Bass kernel for nn_Attention_39651138076722.

ChannelLayerNorm -> qkv 1x1 conv -> 4-head spatial attention (N=4096, dh=32)
-> proj 1x1 conv -> residual.   B=4, C=128, H=W=64.

Sharding: 8 cores = 4 batches x 2 head-pairs. Each core computes the partial
proj output of its 2 heads for its batch; the host sums the two partials.
LayerNorm affine (norm_w/norm_b) is folded into the qkv weights on the host.

Attention: S^T = k.T q as 33-row f32r matmuls (32 k-rows scaled by
SCALE*log2e + one constant "magic" row adding M = 6144 + D_HI*2^-11), so PSUM
holds w = log2(P) + M.  exp is split across two engines by group:
  ACT:  P = Exp(w*ln2 - M*ln2)                      (true exp)
  DVE:  P = bitcast((bits(w) << 12) | D_LO)         (Schraudolph int trick)
The magic-add makes the float w carry round(t*2^11) in its mantissa, so one
int32 shift+or rebuilds the Schraudolph exponent/mantissa approximation
(~3% max rel err, cancels in softmax normalization; end-to-end ~8e-4).
PV accumulates v rows + a ones-row (denominator) per head into one PSUM bank
(heads at partitions 0 and 64).  The group loop is software-pipelined: PV of
group g is emitted after S of group g+1 so the in-order PE queue never waits
on exp at the head of line.  LayerNorm 1/std runs as Exp(-0.5*Ln(var+eps)) on
ACT — same activation table as the attention exp, so the table loads once.
Pool runs the LN elementwise chain + psum->sbuf converts + normalization
muls; the softmax reciprocal broadcast uses one DVE stream_shuffle instead of
a DRAM round-trip.
"""
import sys
sys.path.insert(0, "/opt/trn_rl_repo")

import numpy as np
import concourse.bass as bass
import concourse.tile as tile
from concourse import bacc, mybir
from concourse.bass_utils import run_bass_kernel_spmd

F32 = mybir.dt.float32
F32R = mybir.dt.float32r
BF16 = mybir.dt.bfloat16
I32 = mybir.dt.int32
AF = mybir.ActivationFunctionType
OP = mybir.AluOpType

B, C, H, W = 4, 128, 64, 64
N = H * W                      # 4096
NH, DH = 4, 32
EPS = 1e-6
NCH = 512                      # free-dim chunk (psum bank)
NJ = N // NCH                  # 8 n-chunks
MC = 128                       # m-chunk (partition tile)
NM = N // MC                   # 32 m-chunks
SCALE = DH ** -0.5
LOG2E = 1.4426950408889634
LN2 = 0.6931471805599453

SIGMA = 0.0430                 # Schraudolph bias (centers max rel err ~±3%)
D_CONST = round((127 - SIGMA) * (1 << 23))
D_HI, D_LO = D_CONST >> 12, D_CONST & 0xFFF
MAGIC = 6144.0 + D_HI * 2.0 ** -11   # exact in bf16x2 (f32r stationary)

PAIRS = [(i % 2, i // 2) for i in range(2 * NM)]    # (head, m-chunk)
NP = 2 * NM                                         # 64 pairs per n-chunk
# exp engine per pair: 30 DVE int-trick, 34 ACT true exp, interleaved
EXP_SCHED = ['D' if (i * 30) % NP < 30 else 'A' for i in range(NP)]
PV_LAG = 5                                          # pairs of S->PV software pipeline


def build_nc(reps: int = 1):
    nc = bacc.Bacc("TRN2", target_bir_lowering=False)
    d_x = nc.dram_tensor("x", [C, N], F32R, kind="ExternalInput")
    d_wq = nc.dram_tensor("wq", [C, 96], F32R, kind="ExternalInput")
    d_wk = nc.dram_tensor("wk", [C, 96], F32R, kind="ExternalInput")
    d_wv = nc.dram_tensor("wv", [C, 64], F32R, kind="ExternalInput")
    d_bq = nc.dram_tensor("bq", [96, 1], F32, kind="ExternalInput")
    d_bk = nc.dram_tensor("bk", [96, 1], F32, kind="ExternalInput")
    d_bv = nc.dram_tensor("bv", [C, 64], F32, kind="ExternalInput")
    d_pw = nc.dram_tensor("pw", [65, C], F32R, kind="ExternalInput")  # projT + bias row
    d_res = nc.dram_tensor("res", [C, 1], F32, kind="ExternalInput")  # residual scale col
    d_out = nc.dram_tensor("out", [C, N], F32, kind="ExternalOutput")

    with tile.TileContext(nc) as tc:
        with tc.tile_pool(name="persist", bufs=1) as P:
            x_sb = P.tile([C, N], F32R, tag="x_sb")
            xhat = P.tile([C, N], F32R, tag="xhat")
            qq2 = P.tile([C, N], F32R, tag="qq2")   # 0:32 q_h0, 32 ones, 64:96 q_h1, 96 ones
            kk2 = P.tile([C, N], F32R, tag="kk2")   # 0:32 k_h0*s, 32 MAGIC, 64:96 k_h1*s, 96 MAGIC
            vta = P.tile([C, NM, 66], BF16, tag="vta")   # per m-chunk: [v0|1|v1|1]
            wr_q = P.tile([C, 96], F32R, tag="wr_q")
            wr_k = P.tile([C, 96], F32R, tag="wr_k")
            wr_v = P.tile([C, 64], F32R, tag="wr_v")
            b_q = P.tile([96, 1], F32, tag="b_q")
            b_k = P.tile([96, 1], F32, tag="b_k")
            bv_b = P.tile([C, 64], F32, tag="bv_b")
            wr_p = P.tile([65, C], F32R, tag="wr_p")
            res_c = P.tile([C, 1], F32, tag="res_c")
            ones_m = P.tile([C, C], F32, tag="ones_m")
            ones_r = P.tile([C, C], F32R, tag="ones_r")
            eps_c = P.tile([C, 1], F32, tag="eps_c")
            bexp = P.tile([C, 1], F32, tag="bexp")
            ones_n = P.tile([1, NCH], F32, tag="ones_n")
            one_row = P.tile([1, N], F32, tag="one_row")
            mag_row = P.tile([1, N], F32, tag="mag_row")
            zro = P.tile([1, NCH], F32, tag="zro")
            zro_r = P.tile([1, NCH], F32R, tag="zro_r")
            rec = P.tile([64, NCH], F32, tag="rec")
            hn_a = P.tile([65, NCH], F32R, tag="hn_a")   # hn + ones row (proj bias)
            hn_b = P.tile([65, NCH], F32R, tag="hn_b")

            nc.sync.dma_start(out=wr_q, in_=d_wq.ap())
            nc.sync.dma_start(out=wr_k, in_=d_wk.ap())
            nc.sync.dma_start(out=wr_v, in_=d_wv.ap())
            nc.sync.dma_start(out=b_q, in_=d_bq.ap())
            nc.sync.dma_start(out=b_k, in_=d_bk.ap())
            nc.sync.dma_start(out=bv_b, in_=d_bv.ap())
            nc.sync.dma_start(out=wr_p, in_=d_pw.ap())
            nc.sync.dma_start(out=res_c, in_=d_res.ap())
            nc.vector.memset(ones_m, 1.0)
            nc.vector.memset(eps_c, EPS)
            nc.vector.memset(bexp, -MAGIC * LN2)
            nc.vector.memset(ones_n, 1.0)
            nc.gpsimd.memset(one_row, 1.0)
            nc.gpsimd.memset(mag_row, MAGIC)
            nc.gpsimd.memset(zro, 0.0)
            nc.gpsimd.tensor_copy(out=zro_r, in_=zro)
            nc.vector.memset(rec, 0.0)
            nc.vector.tensor_copy(out=ones_r, in_=ones_m)
            nc.vector.tensor_copy(out=qq2[32:33, :], in_=one_row)
            nc.vector.tensor_copy(out=qq2[96:97, :], in_=one_row)
            nc.gpsimd.tensor_copy(out=kk2[32:33, :], in_=mag_row)
            nc.gpsimd.tensor_copy(out=kk2[96:97, :], in_=mag_row)
            nc.gpsimd.tensor_copy(out=hn_a[64:65, :], in_=ones_n)
            nc.gpsimd.tensor_copy(out=hn_b[64:65, :], in_=ones_n)
            nc.gpsimd.tensor_copy(out=vta[:, :, 32:33], in_=ones_r[:, 0:NM])
            nc.gpsimd.tensor_copy(out=vta[:, :, 65:66], in_=ones_r[:, 0:NM])

            for rep in range(reps):
                with tc.tile_pool(name="stats", bufs=3) as SP, \
                     tc.tile_pool(name="spool", bufs=6, space="PSUM") as SPOOL, \
                     tc.tile_pool(name="pvpool", bufs=2, space="PSUM") as PVP, \
                     tc.tile_pool(name="ptpool", bufs=6) as PTP, \
                     tc.tile_pool(name="opool", bufs=2) as OPO, \
                     tc.tile_pool(name="npool", bufs=2) as NPO:
                    pvs, rbs, invs, mBs = {}, {}, {}, {}
                    pend = []
                    next_p = [0]

                    def emit_pv(item):
                        h, mc, pi, pt, pv = item
                        vcols = slice(33 * h, 33 * h + 33)
                        nc.tensor.matmul(pv[64 * h:64 * h + 33, :],
                                         vta[:, mc, vcols], pt,
                                         start=False,
                                         stop=(pi >= 2 * NM - 2),
                                         skip_group_check=True,
                                         tile_position=(0, 64 * h))

                    def attn_pair(j, pi, pv):
                        js = slice(j * NCH, (j + 1) * NCH)
                        h, mc = PAIRS[pi]
                        sg = SPOOL.tile([C, NCH], F32, tag="sg",
                                        name=f"sg{j}_{pi}")
                        ms = slice(mc * MC, (mc + 1) * MC)
                        rs = slice(64 * h, 64 * h + 33)
                        nc.tensor.matmul(sg, kk2[rs, ms], qq2[rs, js],
                                         start=True, stop=True,
                                         tile_position=(64 * h, 0))
                        # pt holds P in the odd bf16 lanes of [C, 2*NCH]: the
                        # int trick writes int32 whose top half IS the bf16
                        # value; ACT writes the odd lanes directly.
                        pt = PTP.tile([C, 2 * NCH], BF16, tag="pt",
                                      name=f"pt{j}_{pi}")
                        pt_odd = pt.rearrange("p (a b) -> p a b", b=2)[:, :, 1:2]
                        if EXP_SCHED[pi] == 'A':
                            nc.scalar.activation(out=pt_odd, in_=sg,
                                                 func=AF.Exp, scale=LN2,
                                                 bias=bexp)
                        else:
                            nc.vector.tensor_scalar(
                                out=pt.bitcast(I32), in0=sg.bitcast(I32),
                                scalar1=12, scalar2=D_LO,
                                op0=OP.logical_shift_left, op1=OP.bitwise_or)
                        pend.append((h, mc, pi, pt_odd, pv))
                        if len(pend) > PV_LAG:
                            emit_pv(pend.pop(0))

                    def flush_pv():
                        while pend:
                            emit_pv(pend.pop(0))

                    def stream_j0(c, pv0):
                        lim = min(NP, 8 * c + 8)
                        while next_p[0] < lim:
                            attn_pair(0, next_p[0], pv0)
                            next_p[0] += 1

                    def stats(j):
                        js = slice(j * NCH, (j + 1) * NCH)
                        x2 = SP.tile([C, NCH], F32R, tag="x2", name=f"x2_{j}")
                        nc.scalar.activation(out=x2, in_=x_sb[:, js],
                                             func=AF.Square, scale=1.0)
                        s1 = SPOOL.tile([C, NCH], F32, tag="sg", name=f"s1_{j}")
                        nc.tensor.matmul(s1, ones_r, x_sb[:, js],
                                         start=True, stop=True)
                        s2 = SPOOL.tile([C, NCH], F32, tag="sg", name=f"s2_{j}")
                        nc.tensor.matmul(s2, ones_r, x2,
                                         start=True, stop=True)
                        mB = SP.tile([C, NCH], F32, tag="mB", name=f"mB_{j}")
                        nc.scalar.activation(out=mB, in_=s1, func=AF.Copy,
                                             scale=1.0 / C)
                        mBs[j] = mB
                        msq = SP.tile([C, NCH], F32, tag="msq", name=f"msq_{j}")
                        nc.gpsimd.tensor_tensor(out=msq, in0=mB, in1=mB, op=OP.mult)
                        s2n = SP.tile([C, NCH], F32, tag="s2n", name=f"s2n_{j}")
                        nc.vector.tensor_scalar(out=s2n, in0=s2,
                                                scalar1=1.0 / C, scalar2=None,
                                                op0=OP.mult)
                        var = SP.tile([C, NCH], F32, tag="var", name=f"var_{j}")
                        nc.gpsimd.tensor_tensor(out=var, in0=s2n, in1=msq,
                                                op=OP.subtract)
                        lnv = SP.tile([C, NCH], F32, tag="lnv", name=f"lnv_{j}")
                        nc.scalar.activation(out=lnv, in_=var, func=AF.Ln,
                                             bias=eps_c, scale=1.0)
                        inv = SP.tile([C, NCH], F32, tag="inv", name=f"inv_{j}")
                        nc.scalar.activation(out=inv, in_=lnv, func=AF.Exp,
                                             scale=-0.5)
                        invs[j] = inv

                    def pass_b(j):
                        js = slice(j * NCH, (j + 1) * NCH)
                        cen = SP.tile([C, NCH], F32, tag="cen", name=f"cen_{j}")
                        nc.gpsimd.tensor_tensor(out=cen, in0=x_sb[:, js],
                                                in1=mBs[j], op=OP.subtract)
                        nc.gpsimd.tensor_tensor(out=xhat[:, js], in0=cen,
                                                in1=invs[j], op=OP.mult)
                        qp = SPOOL.tile([96, NCH], F32, tag="sg", name=f"qp{j}")
                        nc.tensor.matmul(qp, wr_q, xhat[:, js],
                                         start=True, stop=True)
                        nc.scalar.activation(out=qq2[0:32, js], in_=qp[0:32, :],
                                             func=AF.Identity, bias=b_q[0:32, :],
                                             scale=1.0)
                        nc.scalar.activation(out=qq2[64:96, js], in_=qp[64:96, :],
                                             func=AF.Identity, bias=b_q[64:96, :],
                                             scale=1.0)
                        kp = SPOOL.tile([96, NCH], F32, tag="sg", name=f"kp{j}")
                        nc.tensor.matmul(kp, wr_k, xhat[:, js],
                                         start=True, stop=True)
                        nc.vector.tensor_scalar(out=kk2[0:32, js], in0=kp[0:32, :],
                                                scalar1=b_k[0:32, :], scalar2=None,
                                                op0=OP.add)
                        nc.vector.tensor_scalar(out=kk2[64:96, js], in0=kp[64:96, :],
                                                scalar1=b_k[64:96, :], scalar2=None,
                                                op0=OP.add)
                        vpq = SPOOL.tile([C, 4, 64], F32, tag="sg", name=f"vpq{j}")
                        for mq in range(4):
                            mc = 4 * j + mq
                            ms = slice(mc * MC, (mc + 1) * MC)
                            nc.tensor.matmul(vpq[:, mq, :], xhat[:, ms], wr_v,
                                             start=True, stop=True)
                            vdst = vta[:, mc, 0:66].rearrange(
                                "p (a b) -> p a b", a=2)[:, :, 0:32]
                            vsrc = vpq[:, mq, :].rearrange("p (a b) -> p a b", a=2)
                            bsrc = bv_b.rearrange("p (a b) -> p a b", a=2)
                            nc.vector.tensor_tensor(out=vdst, in0=vsrc, in1=bsrc,
                                                    op=OP.add)

                    def tail1(j):
                        pv = pvs[j]
                        nc.vector.reciprocal(out=rec[0:1, :], in_=pv[32:33, :])
                        nc.vector.reciprocal(out=rec[32:33, :], in_=pv[96:97, :])
                        rb = NPO.tile([64, NCH], F32, tag="rb", name=f"rb{j}")
                        nc.vector.stream_shuffle(out=rb, in_=rec, mask=[0] * 32)
                        rbs[j] = rb

                    def tail2(j):
                        pv, rb = pvs[j], rbs[j]
                        hn = hn_a if j % 2 == 0 else hn_b
                        nc.vector.tensor_tensor(out=hn[0:32, :], in0=pv[0:32, :],
                                                in1=rb[0:32, :], op=OP.mult)
                        nc.vector.tensor_tensor(out=hn[32:64, :], in0=pv[64:96, :],
                                                in1=rb[32:64, :], op=OP.mult)

                    def tail3(j):
                        js = slice(j * NCH, (j + 1) * NCH)
                        hn = hn_a if j % 2 == 0 else hn_b
                        pj = SPOOL.tile([C, NCH], F32, tag="sg", name=f"pj{j}")
                        nc.tensor.matmul(pj, wr_p, hn, start=True, stop=True)
                        ot = OPO.tile([C, NCH], F32, tag="ot", name=f"ot{j}")
                        nc.vector.scalar_tensor_tensor(out=ot, in0=x_sb[:, js],
                                                       scalar=res_c, in1=pj,
                                                       op0=OP.mult, op1=OP.add)
                        nc.sync.dma_start(out=d_out.ap()[:, js], in_=ot)

                    # ---- schedule ----
                    for j in range(NJ):
                        js = slice(j * NCH, (j + 1) * NCH)
                        nc.sync.dma_start(out=x_sb[:, js], in_=d_x.ap()[:, js])
                    def pv_zero(pv):
                        # open one accumulation group covering the whole bank
                        # (both heads' partition bands) with a zeroing matmul
                        nc.tensor.matmul(pv, ones_r[0:1, 0:97], zro_r,
                                         start=True, stop=False,
                                         skip_group_check=True)

                    pv0 = PVP.tile([97, NCH], F32, tag="pv", name="pv_0")
                    pvs[0] = pv0
                    pv_zero(pv0)
                    for c in range(NJ):
                        stats(c)
                        if c >= 1:
                            pass_b(c - 1)
                            stream_j0(c - 1, pv0)
                    pass_b(NJ - 1)
                    stream_j0(NJ - 1, pv0)

                    for j in range(1, NJ):
                        pv = PVP.tile([97, NCH], F32, tag="pv", name=f"pv_{j}")
                        pvs[j] = pv
                        pv_zero(pv)
                        for pi in range(NP):
                            attn_pair(j, pi, pv)
                            if pi == PV_LAG + 1:
                                tail1(j - 1)
                            if pi == PV_LAG + 12:
                                tail2(j - 1)
                            if pi == PV_LAG + 20:
                                tail3(j - 1)
                    flush_pv()
                    tail1(NJ - 1)
                    tail2(NJ - 1)
                    tail3(NJ - 1)
    nc.compile()
    return nc


def _prep_inputs(x, norm_w, norm_b, qkv_w, qkv_b, proj_w, proj_b):
    """Host-side fold + per-core slicing. Returns list of 8 in_maps."""
    xf = np.ascontiguousarray(x.reshape(B, C, N), dtype=np.float32)
    qkv_wf = (qkv_w * norm_w[None, :]).astype(np.float32)
    qkv_bf = (qkv_b + qkv_w @ norm_b).astype(np.float32)
    ks = np.float32(SCALE * LOG2E)
    in_maps = []
    for core in range(8):
        b, hp = core // 2, core % 2
        h0, h1 = 2 * hp, 2 * hp + 1
        q0 = list(range(h0 * DH, h0 * DH + DH))
        q1 = list(range(h1 * DH, h1 * DH + DH))
        k0 = [C + r for r in q0]
        k1 = [C + r for r in q1]
        vrows = [2 * C + r for r in q0 + q1]
        wq = np.zeros((C, 96), np.float32)
        wq[:, 0:32] = qkv_wf[q0, :].T
        wq[:, 64:96] = qkv_wf[q1, :].T
        wk = np.zeros((C, 96), np.float32)
        wk[:, 0:32] = qkv_wf[k0, :].T * ks
        wk[:, 64:96] = qkv_wf[k1, :].T * ks
        bq = np.zeros((96, 1), np.float32)
        bq[0:32, 0] = qkv_bf[q0]
        bq[64:96, 0] = qkv_bf[q1]
        bk = np.zeros((96, 1), np.float32)
        bk[0:32, 0] = qkv_bf[k0] * ks
        bk[64:96, 0] = qkv_bf[k1] * ks
        wv = qkv_wf[vrows, :].T.copy()                   # [C, 64]
        bv = np.broadcast_to(qkv_bf[vrows].reshape(1, 64), (C, 64)).copy()
        cols = q0 + q1
        pw = np.zeros((65, C), np.float32)
        pw[0:64, :] = proj_w[:, cols].T
        if hp == 0:
            pw[64, :] = proj_b
        res = np.full((C, 1), 1.0 if hp == 0 else 0.0, np.float32)
        in_maps.append({
            "x": np.ascontiguousarray(xf[b]), "wq": wq, "wk": wk, "wv": wv,
            "bq": bq, "bk": bk, "bv": bv, "pw": pw, "res": res,
        })
    return in_maps


_NC_CACHE = None


def kernel(x, norm_w, norm_b, qkv_w, qkv_b, proj_w, proj_b, **extra):
    global _NC_CACHE
    x = np.asarray(x, dtype=np.float32)
    in_maps = _prep_inputs(x, np.asarray(norm_w), np.asarray(norm_b),
                           np.asarray(qkv_w), np.asarray(qkv_b),
                           np.asarray(proj_w), np.asarray(proj_b))
    if _NC_CACHE is None:
        _NC_CACHE = build_nc()
    res = run_bass_kernel_spmd(_NC_CACHE, in_maps, core_ids=list(range(8)))
    parts = [res.results[i]["out"] for i in range(8)]
    out = np.empty((B, C, N), np.float32)
    for b in range(B):
        out[b] = parts[2 * b] + parts[2 * b + 1]
    return out.reshape(B, C, H, W)


if __name__ == "__main__":
    rng = np.random.default_rng(0)
    x = rng.standard_normal((B, C, H, W)).astype(np.float32)
    nw = np.ones(C, np.float32)
    nb = np.zeros(C, np.float32)
    qw = (rng.standard_normal((3 * C, C)) / np.sqrt(C)).astype(np.float32)
    qb = np.zeros(3 * C, np.float32)
    pw = (rng.standard_normal((C, C)) / np.sqrt(C)).astype(np.float32)
    pb = np.zeros(C, np.float32)
    got = kernel(x, nw, nb, qw, qb, pw, pb)
    print("kernel ran, shape", got.shape)


# revision 58
# speedup vs baseline: 1.0491x; 1.0491x over previous
"""Trainium2 Bass kernel for nn_Attention_39651138076722.

ChannelLayerNorm -> qkv 1x1 conv -> 4-head spatial attention (N=4096, dh=32)
-> proj 1x1 conv -> residual.   B=4, C=128, H=W=64.

Sharding: 8 cores = 4 batches x 2 head-pairs. Each core computes the partial
proj output of its 2 heads for its batch; the host sums the two partials.
LayerNorm affine (norm_w/norm_b) is folded into the qkv weights on the host.

Attention: S^T = k.T q as 33-row f32r matmuls (32 k-rows scaled by
SCALE*log2e + one constant "magic" row adding M = 6144 + D_HI*2^-11), so PSUM
holds w = log2(P) + M.  exp is split across two engines by group:
  ACT:  P = Exp(w*ln2 - M*ln2)                      (true exp)
  DVE:  P = bitcast((bits(w) << 12) | D_LO)         (Schraudolph int trick)
The magic-add makes the float w carry round(t*2^11) in its mantissa, so one
int32 shift+or rebuilds the Schraudolph exponent/mantissa approximation
(~3% max rel err, cancels in softmax normalization; end-to-end ~8e-4).
PV accumulates v rows + a ones-row (denominator) per head into one PSUM bank
(heads at partitions 0 and 64).  The group loop is software-pipelined: PV of
group g is emitted after S of group g+1 so the in-order PE queue never waits
on exp at the head of line.  LayerNorm 1/std runs as Exp(-0.5*Ln(var+eps)) on
ACT — same activation table as the attention exp, so the table loads once.
Pool runs the LN elementwise chain + psum->sbuf converts + normalization
muls; the softmax reciprocal broadcast uses one DVE stream_shuffle instead of
a DRAM round-trip.
"""
import sys
sys.path.insert(0, "/opt/trn_rl_repo")

import numpy as np
import concourse.bass as bass
import concourse.tile as tile
from concourse import bacc, mybir
from concourse.bass_utils import run_bass_kernel_spmd

F32 = mybir.dt.float32
F32R = mybir.dt.float32r
BF16 = mybir.dt.bfloat16
I32 = mybir.dt.int32
AF = mybir.ActivationFunctionType
OP = mybir.AluOpType

B, C, H, W = 4, 128, 64, 64
N = H * W                      # 4096
NH, DH = 4, 32
EPS = 1e-6
NCH = 512                      # free-dim chunk (psum bank)
NJ = N // NCH                  # 8 n-chunks
MC = 128                       # m-chunk (partition tile)
NM = N // MC                   # 32 m-chunks
SCALE = DH ** -0.5
LOG2E = 1.4426950408889634
LN2 = 0.6931471805599453

SIGMA = 0.0430                 # Schraudolph bias (centers max rel err ~±3%)
D_CONST = round((127 - SIGMA) * (1 << 23))
D_HI, D_LO = D_CONST >> 12, D_CONST & 0xFFF
MAGIC = 6144.0 + D_HI * 2.0 ** -11   # exact in bf16x2 (f32r stationary)

PAIRS = [(i % 2, i // 2) for i in range(2 * NM)]    # (head, m-chunk)
NP = 2 * NM                                         # 64 pairs per n-chunk
# exp engine per pair: DVE int-trick vs ACT true exp, interleaved.
# j0/j1 stream during the LN/proj phase where ACT carries the stats chain ->
# bias their exps toward DVE; steady js lean back on ACT.
def _sched(nd, head_a=0):
    n = NP - head_a
    return ['A'] * head_a + ['D' if (i * nd) % n < nd else 'A' for i in range(n)]


EXP_SCHED_EARLY = _sched(40)
EXP_SCHED_LATE = _sched(26, head_a=6)
PV_LAG = 5                                          # pairs of S->PV software pipeline


def build_nc(reps: int = 1):
    nc = bacc.Bacc("TRN2", target_bir_lowering=False)
    d_x = nc.dram_tensor("x", [C, N], F32R, kind="ExternalInput")
    d_wq = nc.dram_tensor("wq", [C, 96], F32R, kind="ExternalInput")
    d_wk = nc.dram_tensor("wk", [C, 96], F32R, kind="ExternalInput")
    d_wv = nc.dram_tensor("wv", [C, 64], F32R, kind="ExternalInput")
    d_bq = nc.dram_tensor("bq", [96, 1], F32, kind="ExternalInput")
    d_bk = nc.dram_tensor("bk", [96, 1], F32, kind="ExternalInput")
    d_bv = nc.dram_tensor("bv", [C, 64], F32, kind="ExternalInput")
    d_pw = nc.dram_tensor("pw", [65, C], F32R, kind="ExternalInput")  # projT + bias row
    d_res = nc.dram_tensor("res", [C, 1], F32, kind="ExternalInput")  # residual scale col
    d_rows = nc.dram_tensor("rows", [2, N], F32R, kind="ExternalInput")  # [ones, magic]
    d_out = nc.dram_tensor("out", [C, N], F32, kind="ExternalOutput")

    with tile.TileContext(nc) as tc:
        with tc.tile_pool(name="persist", bufs=1) as P:
            x_sb = P.tile([C, N], F32R, tag="x_sb")
            xhat = P.tile([C, N], F32R, tag="xhat")
            qq2 = P.tile([C, N], F32R, tag="qq2")   # 0:32 q_h0, 32 ones, 64:96 q_h1, 96 ones
            kk2 = P.tile([C, N], F32R, tag="kk2")   # 0:32 k_h0*s, 32 MAGIC, 64:96 k_h1*s, 96 MAGIC
            vta = P.tile([C, NM, 66], BF16, tag="vta")   # per m-chunk: [v0|1|v1|1]
            wr_q = P.tile([C, 96], F32R, tag="wr_q")
            wr_k = P.tile([C, 96], F32R, tag="wr_k")
            wr_v = P.tile([C, 64], F32R, tag="wr_v")
            b_q = P.tile([96, 1], F32, tag="b_q")
            b_k = P.tile([96, 1], F32, tag="b_k")
            bv_b = P.tile([C, 64], F32, tag="bv_b")
            wr_p = P.tile([65, C], F32R, tag="wr_p")
            res_c = P.tile([C, 1], F32, tag="res_c")
            ones_m = P.tile([C, C], F32, tag="ones_m")
            ones_r = P.tile([C, C], F32R, tag="ones_r")
            eps_c = P.tile([C, 1], F32, tag="eps_c")
            bexp = P.tile([C, 1], F32, tag="bexp")
            ones_n = P.tile([1, NCH], F32, tag="ones_n")
            zro = P.tile([1, NCH], F32, tag="zro")
            zro_r = P.tile([1, NCH], F32R, tag="zro_r")
            rec = P.tile([64, NCH], F32, tag="rec")
            hn_a = P.tile([65, NCH], F32R, tag="hn_a")   # hn + ones row (proj bias)
            hn_b = P.tile([65, NCH], F32R, tag="hn_b")

            nc.sync.dma_start(out=wr_q, in_=d_wq.ap())
            nc.sync.dma_start(out=wr_k, in_=d_wk.ap())
            nc.sync.dma_start(out=wr_v, in_=d_wv.ap())
            nc.sync.dma_start(out=b_q, in_=d_bq.ap())
            nc.sync.dma_start(out=b_k, in_=d_bk.ap())
            nc.sync.dma_start(out=bv_b, in_=d_bv.ap())
            nc.sync.dma_start(out=wr_p, in_=d_pw.ap())
            nc.sync.dma_start(out=res_c, in_=d_res.ap())
            nc.vector.memset(ones_m, 1.0)
            nc.vector.memset(eps_c, EPS)
            nc.vector.memset(bexp, -MAGIC * LN2)
            nc.vector.memset(ones_n, 1.0)
            nc.sync.dma_start(out=qq2[32:33, :], in_=d_rows.ap()[0:1, :])
            nc.sync.dma_start(out=qq2[96:97, :], in_=d_rows.ap()[0:1, :])
            nc.sync.dma_start(out=kk2[32:33, :], in_=d_rows.ap()[1:2, :])
            nc.sync.dma_start(out=kk2[96:97, :], in_=d_rows.ap()[1:2, :])
            nc.gpsimd.memset(zro, 0.0)
            nc.gpsimd.tensor_copy(out=zro_r, in_=zro)
            nc.vector.memset(rec, 0.0)
            nc.vector.tensor_copy(out=ones_r, in_=ones_m)
            nc.gpsimd.tensor_copy(out=hn_a[64:65, :], in_=ones_n)
            nc.gpsimd.tensor_copy(out=hn_b[64:65, :], in_=ones_n)
            nc.gpsimd.tensor_copy(out=vta[:, :, 32:33], in_=ones_r[:, 0:NM])
            nc.gpsimd.tensor_copy(out=vta[:, :, 65:66], in_=ones_r[:, 0:NM])

            for rep in range(reps):
                with tc.tile_pool(name="stats", bufs=3) as SP, \
                     tc.tile_pool(name="spool", bufs=6, space="PSUM") as SPOOL, \
                     tc.tile_pool(name="pvpool", bufs=2, space="PSUM") as PVP, \
                     tc.tile_pool(name="ptpool", bufs=6) as PTP, \
                     tc.tile_pool(name="opool", bufs=2) as OPO, \
                     tc.tile_pool(name="npool", bufs=2) as NPO:
                    pvs, rbs, invs, mBs = {}, {}, {}, {}
                    pend = []
                    next_p = [0, 0]

                    def emit_pv(item):
                        h, mc, pi, pt, pv = item
                        vcols = slice(33 * h, 33 * h + 33)
                        nc.tensor.matmul(pv[64 * h:64 * h + 33, :],
                                         vta[:, mc, vcols], pt,
                                         start=False,
                                         stop=(pi >= 2 * NM - 2),
                                         skip_group_check=True,
                                         tile_position=(0, 64 * h))

                    def attn_pair(j, pi, pv):
                        js = slice(j * NCH, (j + 1) * NCH)
                        h, mc = PAIRS[pi]
                        sg = SPOOL.tile([C, NCH], F32, tag="sg",
                                        name=f"sg{j}_{pi}")
                        ms = slice(mc * MC, (mc + 1) * MC)
                        rs = slice(64 * h, 64 * h + 33)
                        nc.tensor.matmul(sg, kk2[rs, ms], qq2[rs, js],
                                         start=True, stop=True,
                                         tile_position=(64 * h, 0))
                        # pt holds P in the odd bf16 lanes of [C, 2*NCH]: the
                        # int trick writes int32 whose top half IS the bf16
                        # value; ACT writes the odd lanes directly.
                        pt = PTP.tile([C, 2 * NCH], BF16, tag="pt",
                                      name=f"pt{j}_{pi}")
                        pt_odd = pt.rearrange("p (a b) -> p a b", b=2)[:, :, 1:2]
                        sched = EXP_SCHED_EARLY if j < 1 else EXP_SCHED_LATE
                        if sched[pi] == 'A':
                            nc.scalar.activation(out=pt_odd, in_=sg,
                                                 func=AF.Exp, scale=LN2,
                                                 bias=bexp)
                        else:
                            nc.vector.tensor_scalar(
                                out=pt.bitcast(I32), in0=sg.bitcast(I32),
                                scalar1=12, scalar2=D_LO,
                                op0=OP.logical_shift_left, op1=OP.bitwise_or)
                        pend.append((h, mc, pi, pt_odd, pv))
                        if len(pend) > PV_LAG:
                            emit_pv(pend.pop(0))

                    def flush_pv():
                        while pend:
                            emit_pv(pend.pop(0))

                    def stream(c):
                        # emit ready pairs of j0 (mc chunks <= c done)
                        lim = min(NP, 8 * c + 8)
                        while next_p[0] < lim:
                            attn_pair(0, next_p[0], pvs[0])
                            next_p[0] += 1

                    def stats(j):
                        js = slice(j * NCH, (j + 1) * NCH)
                        x2 = SP.tile([C, NCH], F32R, tag="x2", name=f"x2_{j}")
                        nc.gpsimd.tensor_tensor(out=x2, in0=x_sb[:, js],
                                                in1=x_sb[:, js], op=OP.mult)
                        s1 = SPOOL.tile([C, NCH], F32, tag="sg", name=f"s1_{j}")
                        nc.tensor.matmul(s1, ones_r, x_sb[:, js],
                                         start=True, stop=True)
                        s2 = SPOOL.tile([C, NCH], F32, tag="sg", name=f"s2_{j}")
                        nc.tensor.matmul(s2, ones_r, x2,
                                         start=True, stop=True)
                        mB = SP.tile([C, NCH], F32, tag="mB", name=f"mB_{j}")
                        nc.scalar.activation(out=mB, in_=s1, func=AF.Copy,
                                             scale=1.0 / C)
                        mBs[j] = mB
                        msq = SP.tile([C, NCH], F32, tag="msq", name=f"msq_{j}")
                        nc.scalar.activation(out=msq, in_=s1, func=AF.Square,
                                             scale=1.0 / C)
                        var = SP.tile([C, NCH], F32, tag="var", name=f"var_{j}")
                        nc.vector.scalar_tensor_tensor(out=var, in0=s2,
                                                       scalar=1.0 / C, in1=msq,
                                                       op0=OP.mult,
                                                       op1=OP.subtract)
                        lnv = SP.tile([C, NCH], F32, tag="lnv", name=f"lnv_{j}")
                        nc.scalar.activation(out=lnv, in_=var, func=AF.Ln,
                                             bias=eps_c, scale=1.0)
                        inv = SP.tile([C, NCH], F32, tag="inv", name=f"inv_{j}")
                        nc.scalar.activation(out=inv, in_=lnv, func=AF.Exp,
                                             scale=-0.5)
                        invs[j] = inv

                    def pass_b(j):
                        js = slice(j * NCH, (j + 1) * NCH)
                        cen = SP.tile([C, NCH], F32, tag="cen", name=f"cen_{j}")
                        nc.gpsimd.tensor_tensor(out=cen, in0=x_sb[:, js],
                                                in1=mBs[j], op=OP.subtract)
                        nc.gpsimd.tensor_tensor(out=xhat[:, js], in0=cen,
                                                in1=invs[j], op=OP.mult)
                        qp = SPOOL.tile([96, NCH], F32, tag="sg", name=f"qp{j}")
                        nc.tensor.matmul(qp, wr_q, xhat[:, js],
                                         start=True, stop=True)
                        nc.scalar.activation(out=qq2[0:32, js], in_=qp[0:32, :],
                                             func=AF.Identity, bias=b_q[0:32, :],
                                             scale=1.0)
                        nc.scalar.activation(out=qq2[64:96, js], in_=qp[64:96, :],
                                             func=AF.Identity, bias=b_q[64:96, :],
                                             scale=1.0)
                        kp = SPOOL.tile([96, NCH], F32, tag="sg", name=f"kp{j}")
                        nc.tensor.matmul(kp, wr_k, xhat[:, js],
                                         start=True, stop=True)
                        nc.vector.tensor_scalar(out=kk2[0:32, js], in0=kp[0:32, :],
                                                scalar1=b_k[0:32, :], scalar2=None,
                                                op0=OP.add)
                        nc.vector.tensor_scalar(out=kk2[64:96, js], in0=kp[64:96, :],
                                                scalar1=b_k[64:96, :], scalar2=None,
                                                op0=OP.add)
                        vpq = SPOOL.tile([C, 4, 64], F32, tag="sg", name=f"vpq{j}")
                        for mq in range(4):
                            mc = 4 * j + mq
                            ms = slice(mc * MC, (mc + 1) * MC)
                            nc.tensor.matmul(vpq[:, mq, :], xhat[:, ms], wr_v,
                                             start=True, stop=True)
                            vdst = vta[:, mc, 0:66].rearrange(
                                "p (a b) -> p a b", a=2)[:, :, 0:32]
                            vsrc = vpq[:, mq, :].rearrange("p (a b) -> p a b", a=2)
                            bsrc = bv_b.rearrange("p (a b) -> p a b", a=2)
                            nc.vector.tensor_tensor(out=vdst, in0=vsrc, in1=bsrc,
                                                    op=OP.add)

                    def tail1(j):
                        pv = pvs[j]
                        nc.vector.reciprocal(out=rec[0:1, :], in_=pv[32:33, :])
                        nc.vector.reciprocal(out=rec[32:33, :], in_=pv[96:97, :])
                        rb = NPO.tile([64, NCH], F32, tag="rb", name=f"rb{j}")
                        nc.vector.stream_shuffle(out=rb, in_=rec, mask=[0] * 32)
                        rbs[j] = rb

                    def tail2(j):
                        pv, rb = pvs[j], rbs[j]
                        hn = hn_a if j % 2 == 0 else hn_b
                        nc.vector.tensor_tensor(out=hn[0:32, :], in0=pv[0:32, :],
                                                in1=rb[0:32, :], op=OP.mult)
                        nc.vector.tensor_tensor(out=hn[32:64, :], in0=pv[64:96, :],
                                                in1=rb[32:64, :], op=OP.mult)

                    def tail3(j):
                        js = slice(j * NCH, (j + 1) * NCH)
                        hn = hn_a if j % 2 == 0 else hn_b
                        pj = PVP.tile([C, NCH], F32, tag="pv", name=f"pj{j}")
                        nc.tensor.matmul(pj, wr_p, hn, start=True, stop=True)
                        ot = OPO.tile([C, NCH], F32, tag="ot", name=f"ot{j}")
                        nc.vector.scalar_tensor_tensor(out=ot, in0=x_sb[:, js],
                                                       scalar=res_c, in1=pj,
                                                       op0=OP.mult, op1=OP.add)
                        nc.sync.dma_start(out=d_out.ap()[:, js], in_=ot)

                    # ---- schedule ----
                    for j in range(NJ):
                        js = slice(j * NCH, (j + 1) * NCH)
                        nc.sync.dma_start(out=x_sb[:, js], in_=d_x.ap()[:, js])
                    def pv_zero(pv):
                        # open one accumulation group covering the whole bank
                        # (both heads' partition bands) with a zeroing matmul
                        nc.tensor.matmul(pv, ones_r[0:1, 0:97], zro_r,
                                         start=True, stop=False,
                                         skip_group_check=True)

                    pv0 = PVP.tile([97, NCH], F32, tag="pv", name="pv_0")
                    pvs[0] = pv0
                    pv_zero(pv0)
                    for c in range(NJ + 2):
                        if c < NJ:
                            stats(c)
                        if c >= 2:
                            pass_b(c - 2)
                            stream(c - 2)

                    for j in range(1, NJ):
                        flush_pv()
                        tail1(j - 1)
                        tail2(j - 1)
                        pv = PVP.tile([97, NCH], F32, tag="pv", name=f"pv_{j}")
                        pvs[j] = pv
                        pv_zero(pv)
                        for pi in range(NP):
                            attn_pair(j, pi, pv)
                            if pi == 8:
                                tail3(j - 1)
                    flush_pv()
                    tail1(NJ - 1)
                    tail2(NJ - 1)
                    tail3(NJ - 1)
    nc.compile()
    return nc


def _prep_inputs(x, norm_w, norm_b, qkv_w, qkv_b, proj_w, proj_b):
    """Host-side fold + per-core slicing. Returns list of 8 in_maps."""
    xf = np.ascontiguousarray(x.reshape(B, C, N), dtype=np.float32)
    qkv_wf = (qkv_w * norm_w[None, :]).astype(np.float32)
    qkv_bf = (qkv_b + qkv_w @ norm_b).astype(np.float32)
    ks = np.float32(SCALE * LOG2E)
    in_maps = []
    for core in range(8):
        b, hp = core // 2, core % 2
        h0, h1 = 2 * hp, 2 * hp + 1
        q0 = list(range(h0 * DH, h0 * DH + DH))
        q1 = list(range(h1 * DH, h1 * DH + DH))
        k0 = [C + r for r in q0]
        k1 = [C + r for r in q1]
        vrows = [2 * C + r for r in q0 + q1]
        wq = np.zeros((C, 96), np.float32)
        wq[:, 0:32] = qkv_wf[q0, :].T
        wq[:, 64:96] = qkv_wf[q1, :].T
        wk = np.zeros((C, 96), np.float32)
        wk[:, 0:32] = qkv_wf[k0, :].T * ks
        wk[:, 64:96] = qkv_wf[k1, :].T * ks
        bq = np.zeros((96, 1), np.float32)
        bq[0:32, 0] = qkv_bf[q0]
        bq[64:96, 0] = qkv_bf[q1]
        bk = np.zeros((96, 1), np.float32)
        bk[0:32, 0] = qkv_bf[k0] * ks
        bk[64:96, 0] = qkv_bf[k1] * ks
        wv = qkv_wf[vrows, :].T.copy()                   # [C, 64]
        bv = np.broadcast_to(qkv_bf[vrows].reshape(1, 64), (C, 64)).copy()
        cols = q0 + q1
        pw = np.zeros((65, C), np.float32)
        pw[0:64, :] = proj_w[:, cols].T
        if hp == 0:
            pw[64, :] = proj_b
        res = np.full((C, 1), 1.0 if hp == 0 else 0.0, np.float32)
        rows = np.empty((2, N), np.float32)
        rows[0] = 1.0
        rows[1] = MAGIC
        in_maps.append({
            "x": np.ascontiguousarray(xf[b]), "wq": wq, "wk": wk, "wv": wv,
            "bq": bq, "bk": bk, "bv": bv, "pw": pw, "res": res, "rows": rows,
        })
    return in_maps


_NC_CACHE = None


def kernel(x, norm_w, norm_b, qkv_w, qkv_b, proj_w, proj_b, **extra):
    global _NC_CACHE
    x = np.asarray(x, dtype=np.float32)
    in_maps = _prep_inputs(x, np.asarray(norm_w), np.asarray(norm_b),
                           np.asarray(qkv_w), np.asarray(qkv_b),
                           np.asarray(proj_w), np.asarray(proj_b))
    if _NC_CACHE is None:
        _NC_CACHE = build_nc()
    res = run_bass_kernel_spmd(_NC_CACHE, in_maps, core_ids=list(range(8)))
    parts = [res.results[i]["out"] for i in range(8)]
    out = np.empty((B, C, N), np.float32)
    for b in range(B):
        out[b] = parts[2 * b] + parts[2 * b + 1]
    return out.reshape(B, C, H, W)


if __name__ == "__main__":
    rng = np.random.default_rng(0)
    x = rng.standard_normal((B, C, H, W)).astype(np.float32)
    nw = np.ones(C, np.float32)
    nb = np.zeros(C, np.float32)
    qw = (rng.standard_normal((3 * C, C)) / np.sqrt(C)).astype(np.float32)
    qb = np.zeros(3 * C, np.float32)
    pw = (rng.standard_normal((C, C)) / np.sqrt(C)).astype(np.float32)
    pb = np.zeros(C, np.float32)
    got = kernel(x, nw, nb, qw, qb, pw, pb)
    print("kernel ran, shape", got.shape)
